# revision 12
# baseline (speedup 1.0000x reference)
"""Trainium2 Bass kernel for nn_Backward_12094627905824 (MLP trunk + gumbel-argmax
mixture sampling). Data-parallel over 8 NeuronCores: batch B=262144 is sharded
32768 rows/core; the small MLP / head weights are replicated.

Math per batch row b (reference semantics):
  h = relu chain: 3 -> 128 -> 256 -> 200
  mu/sig/pai[g,d] = heads (25 comps x 4 dims), pai/sigma through abs
  idx[d] = argmax_g log(pai+1e-12) + gumbel[b,g,d]
  out[b,d] = rand[b,d]*|sig[idx,d]| + mu[idx,d]

On-device reformulation (argmax-invariant): t = pai_raw * exp(gumbel);
score = |t|; selection via one-hot (|t| >= rowmax|t|) mask-and-sum.

Layout: batch row (within a 512-col tile) = p*4 + s where p = psum partition,
s = head-sub index. This makes each gumbel DMA partition line 1600B
contiguous. Head biases ride as two extra "ones" feature rows of h3 (hi/lo
mantissa split), so no separate bias matmuls. Elementwise work is spread
across ACT / DVE / GPSIMD(Pool).
"""
import numpy as np

import concourse.bass as bass
import concourse.mybir as mybir
import bass_rust
from concourse.tile import TileContext
from concourse.bass_utils import run_bass_kernel_spmd

NCORES = 8
B, G, D = 262144, 25, 4
GD = G * D                       # 100
H1, H2, H3 = 128, 256, 200
H3E = H3 + 2                     # +2 ones rows carrying head bias hi/lo
BS = B // NCORES                 # 32768 rows per core
NB = 512                         # batch columns per compute tile
NT = BS // NB                    # 64 tiles
NSUB = NB // 128                 # 4 sub-blocks of 128 rows
TG = 4                           # tiles per gumbel/x DMA group

F32 = mybir.dt.float32
F32R = mybir.dt.float32r
ALU = mybir.AluOpType
AF = mybir.ActivationFunctionType


def _split_multi_waits(nc):
    # walrus CoreV3 codegen accepts only one sync-wait per instruction; Tile's
    # exit drain waits once per active proc. Split into single-wait drains.
    for bb in nc.main_func.blocks:
        insts = list(bb.instructions)
        out = []
        changed = False
        for inst in insts:
            si = inst.sync_info
            if si is not None and len(si.on_wait) > 1:
                waits = list(si.on_wait)
                for k, w in enumerate(waits[:-1]):
                    d = mybir.InstDrain(name=f"{inst.name}-sw{k}", ins=[], outs=[])
                    d.engine = inst.engine
                    d.sync_info = bass_rust.SyncInfo(on_wait=[w], on_update=[])
                    nc.register_instruction(d)
                    out.append(d)
                si.on_wait = [waits[-1]]
                changed = True
            out.append(inst)
        if changed:
            bb.instructions = out


def _build_nc(relu1_split=0):
    nc = bass.Bass(trn_type="TRN2")

    x0t = nc.dram_tensor("x0t", [3, BS], F32R, kind="ExternalInput")
    gum = nc.dram_tensor("gum", [BS, GD], F32, kind="ExternalInput")
    rnd = nc.dram_tensor("rnd", [128, NT * 16], F32, kind="ExternalInput")
    w1t = nc.dram_tensor("w1t", [3, H1], F32R, kind="ExternalInput")
    b1 = nc.dram_tensor("b1", [H1, 1], F32, kind="ExternalInput")
    w2t = nc.dram_tensor("w2t", [H1, H2], F32R, kind="ExternalInput")
    b2s = nc.dram_tensor("b2s", [2, H2], F32R, kind="ExternalInput")   # hi/lo
    w3t = nc.dram_tensor("w3t", [H2, H3E], F32R, kind="ExternalInput")
    b3s = nc.dram_tensor("b3s", [2, H3E], F32R, kind="ExternalInput")  # hi/lo
    wh = nc.dram_tensor("wh", [H3E, 300], F32R, kind="ExternalInput")
    onesr = nc.dram_tensor("onesr", [2, NB], F32R, kind="ExternalInput")
    out_d = nc.dram_tensor("out", [128, NT * 16], F32, kind="ExternalOutput")

    from contextlib import ExitStack
    with TileContext(nc) as tc, ExitStack() as ctx:
        const = ctx.enter_context(tc.tile_pool(name="const", bufs=1))
        io = ctx.enter_context(tc.tile_pool(name="io", bufs=2))
        act = ctx.enter_context(tc.tile_pool(name="act", bufs=2))
        samp = ctx.enter_context(tc.tile_pool(name="samp", bufs=2))
        ptrunk = ctx.enter_context(tc.tile_pool(name="ptrunk", bufs=2, space="PSUM"))
        pheads = ctx.enter_context(tc.tile_pool(name="pheads", bufs=1, space="PSUM"))

        # --- load weights once ---
        w1t_s = const.tile([3, H1], F32R)
        nc.sync.dma_start(out=w1t_s, in_=w1t[:, :])
        b1_s = const.tile([H1, 1], F32)
        nc.sync.dma_start(out=b1_s, in_=b1[:, :])
        w2t_s = const.tile([H1, H2], F32R)
        nc.sync.dma_start(out=w2t_s, in_=w2t[:, :])
        b2s_s = const.tile([2, H2], F32R, tag="b2s")
        nc.sync.dma_start(out=b2s_s, in_=b2s[:, :])
        w3ta_s = const.tile([128, H3E], F32R, tag="w3ta")   # h2 feats 0:128
        nc.sync.dma_start(out=w3ta_s, in_=w3t[0:128, :])
        w3tb_s = const.tile([128, H3E], F32R, tag="w3tb")   # h2 feats 128:256
        nc.sync.dma_start(out=w3tb_s, in_=w3t[128:256, :])
        b3s_s = const.tile([2, H3E], F32R, tag="b3s")
        nc.sync.dma_start(out=b3s_s, in_=b3s[:, :])
        wha_s = const.tile([128, 300], F32R, tag="wha")     # h3 feats 0:128
        nc.sync.dma_start(out=wha_s, in_=wh[0:128, :])
        whb_s = const.tile([74, 300], F32R, tag="whb")      # h3 feats 128:202
        nc.sync.dma_start(out=whb_s, in_=wh[128:202, :])
        rnd_s = const.tile([128, NT, 4, D], F32, tag="rnd")
        nc.sync.dma_start(out=rnd_s, in_=rnd[:, :].rearrange(
            "p (t s d) -> p t s d", t=NT, s=4))
        ones_s = const.tile([2, NB], F32R, tag="ones")
        nc.sync.dma_start(out=ones_s, in_=onesr[:, :])
        out_acc = const.tile([128, NT, 4, D], F32, tag="oacc")

        for it in range(NT):
            jt = it % TG
            if jt == 0:
                r0 = it * NB
                # gumbel: partition p holds rows r0+tile*512+p*4 .. +3
                # (1600B contiguous per tile) for 4 tiles.
                gum_s = io.tile([128, TG, 4, GD], F32, tag="gum")
                nc.sync.dma_start(
                    out=gum_s,
                    in_=gum[r0:r0 + TG * NB, :].rearrange(
                        "(t p s) e -> p t s e", t=TG, p=128),
                )
                x_s = io.tile([3, TG, NB], F32R, tag="x")
                nc.sync.dma_start(
                    out=x_s,
                    in_=x0t[:, r0:r0 + TG * NB].rearrange(
                        "c (t n) -> c t n", t=TG),
                )

            # --- trunk ---
            h1p = ptrunk.tile([128, 2, NB], F32, tag="pt")
            nc.tensor.matmul(h1p[:, 0, :], lhsT=w1t_s[:, :], rhs=x_s[:, jt, :],
                             start=True, stop=True)
            h1 = act.tile([128, NB], F32R, tag="h1")
            c = relu1_split
            if c > 0:
                nc.scalar.activation(h1[:, 0:c], h1p[:, 0, 0:c], func=AF.Relu,
                                     bias=b1_s[:, :], scale=1.0)
                nc.vector.tensor_scalar(h1[:, c:NB], h1p[:, 0, c:NB],
                                        b1_s[:, :], 0.0,
                                        op0=ALU.add, op1=ALU.max)
            else:
                nc.scalar.activation(h1, h1p[:, 0, :], func=AF.Relu,
                                     bias=b1_s[:, :], scale=1.0)

            # psum = W2 h1 + b2 (bias via ones-matmul, hi+lo rows)
            h2p = ptrunk.tile([128, 2, NB], F32, tag="pt")
            nc.tensor.matmul(h2p[:, 0, :], lhsT=w2t_s[:, 0:128], rhs=h1[:, :],
                             start=True, stop=False)
            nc.tensor.matmul(h2p[:, 0, :], lhsT=b2s_s[:, 0:128], rhs=ones_s,
                             start=False, stop=True)
            nc.tensor.matmul(h2p[:, 1, :], lhsT=w2t_s[:, 128:256], rhs=h1[:, :],
                             start=True, stop=False)
            nc.tensor.matmul(h2p[:, 1, :], lhsT=b2s_s[:, 128:256], rhs=ones_s,
                             start=False, stop=True)
            # merged bias-free relu over both halves
            h2ab = act.tile([128, 2, NB], F32R, tag="h2ab")
            nc.scalar.activation(h2ab, h2p, func=AF.Relu, scale=1.0)
            h2a = h2ab[:, 0, :]
            h2b = h2ab[:, 1, :]

            h3p = ptrunk.tile([128, 2, NB], F32, tag="pt")
            nc.tensor.matmul(h3p[:, 0, :], lhsT=w3ta_s[:, 0:128], rhs=h2a,
                             start=True, stop=False)
            nc.tensor.matmul(h3p[:, 0, :], lhsT=w3tb_s[:, 0:128], rhs=h2b,
                             start=False, stop=False)
            nc.tensor.matmul(h3p[:, 0, :], lhsT=b3s_s[:, 0:128], rhs=ones_s,
                             start=False, stop=True)
            nc.tensor.matmul(h3p[0:74, 1, :], lhsT=w3ta_s[:, 128:H3E],
                             rhs=h2a, start=True, stop=False)
            nc.tensor.matmul(h3p[0:74, 1, :], lhsT=w3tb_s[:, 128:H3E],
                             rhs=h2b, start=False, stop=False)
            nc.tensor.matmul(h3p[0:74, 1, :], lhsT=b3s_s[:, 128:H3E],
                             rhs=ones_s, start=False, stop=True)
            # merged bias-free relu; rows 72,73 of half b become exact 1.0
            # (psum there is 0*h2 + 1.0) -- they carry the head biases.
            h3ab = act.tile([128, 2, NB], F32R, tag="h3ab")
            nc.vector.tensor_scalar_max(h3ab, h3p, 0.0)
            h3a = h3ab[:, 0, :]
            h3b = h3ab[0:74, 1, :]

            # --- heads: hp[:, s, 0:300] = [mu(100) | sig(100) | pai(100)] ---
            hp = pheads.tile([128, NSUB, 512], F32, tag="hp")
            for s in range(NSUB):
                c0, c1 = s * 128, (s + 1) * 128
                nc.tensor.matmul(hp[:, s, 0:300], lhsT=h3a[:, c0:c1],
                                 rhs=wha_s[:, :], start=True, stop=False)
                nc.tensor.matmul(hp[:, s, 0:300], lhsT=h3b[:, c0:c1],
                                 rhs=whb_s[:, :], start=False, stop=True)

            # --- sampling ---
            ex = samp.tile([128, 4, GD], F32, tag="ex")
            nc.scalar.activation(ex, gum_s[:, jt], func=AF.Exp, scale=2.0)

            # score = pai_raw^2 * exp(2*gumbel)  (same argmax as |pai|*e^g)
            sq = samp.tile([128, 4, GD], F32, tag="sq")
            nc.scalar.activation(sq, hp[:, :, 200:300], func=AF.Square)
            ts = samp.tile([128, 4, GD], F32, tag="ts")
            nc.gpsimd.tensor_mul(ts, sq, ex)

            # rowmax of score over g per (s, d)
            ts_v = ts.rearrange("p s (g d) -> p s d g", g=G)
            smax = samp.tile([128, 4, D], F32, tag="smax")
            nc.vector.tensor_reduce(smax, ts_v, axis=mybir.AxisListType.X,
                                    op=ALU.max)

            # one-hot: oh = (score >= smax)   [Pool, sbuf only]
            oh = samp.tile([128, 4, GD], F32, tag="oh")
            smax_b = smax.unsqueeze(3).broadcast_to([128, 4, D, G])
            nc.vector.tensor_tensor(
                out=oh.rearrange("p s (g d) -> p s d g", g=G),
                in0=ts_v, in1=smax_b, op=ALU.is_ge)

            # masked select-sum of mu and sig
            selm = samp.tile([128, 4, 2, GD], F32, tag="selm")
            oh_b = oh.unsqueeze(2).broadcast_to([128, 4, 2, GD])
            nc.vector.tensor_mul(
                selm, hp[:, :, 0:200].rearrange("p s (h e) -> p s h e", h=2),
                oh_b)
            sel = samp.tile([128, 4, 2, D], F32, tag="sel")
            nc.vector.tensor_reduce(
                sel, selm.rearrange("p s h (g d) -> p s h d g", g=G),
                axis=mybir.AxisListType.X, op=ALU.add)

            # out = rnd * |sig_sel| + mu_sel   [Pool, sbuf only]
            siga = samp.tile([128, 4, D], F32, tag="siga")
            nc.vector.scalar_tensor_tensor(
                out=siga, in0=sel[:, :, 1, :], scalar=-1.0,
                in1=sel[:, :, 1, :], op0=ALU.mult, op1=ALU.max)
            ot = samp.tile([128, 4, D], F32, tag="ot")
            nc.gpsimd.tensor_mul(ot, rnd_s[:, it], siga)
            nc.gpsimd.tensor_add(out_acc[:, it], ot, sel[:, :, 0, :])

        nc.sync.dma_start(
            out=out_d[:, :],
            in_=out_acc.rearrange("p t s d -> p (t s d)"))

    _split_multi_waits(nc)
    return nc


_NC_CACHE = None
LAST_RESULT = None


def _split10(a):
    """hi = a with mantissa truncated to 10 explicit bits (exactly
    representable in fp32r), lo = exact residual."""
    a = np.ascontiguousarray(a, np.float32)
    hi = (a.view(np.uint32) & np.uint32(0xFFFFE000)).view(np.float32)
    return hi, np.ascontiguousarray(a - hi)


def kernel(x0, rand, gumbel, W1, b1, W2, b2, W3, b3,
           Wmu, bmu, Wsig, bsig, Wpai, bpai):
    global _NC_CACHE, LAST_RESULT
    if _NC_CACHE is None:
        _NC_CACHE = _build_nc()
    nc = _NC_CACHE

    x0 = np.ascontiguousarray(np.asarray(x0, np.float32))
    rand = np.ascontiguousarray(np.asarray(rand, np.float32))
    gumbel = np.ascontiguousarray(np.asarray(gumbel, np.float32))

    # stacked head weights [202, 300]: rows 0..199 = h3 feats,
    # rows 200,201 = bias hi/lo; col = head*100 + g*4 + d
    WH = np.zeros((H3E, 300), np.float32)
    for hd, (W, b) in enumerate([(Wmu, bmu), (Wsig, bsig), (Wpai, bpai)]):
        WH[:H3, hd * GD:(hd + 1) * GD] = np.asarray(W, np.float32).reshape(GD, H3).T
        bhi, blo = _split10(np.asarray(b, np.float32).reshape(1, GD))
        WH[H3, hd * GD:(hd + 1) * GD] = bhi
        WH[H3 + 1, hd * GD:(hd + 1) * GD] = blo

    w3e = np.zeros((H2, H3E), np.float32)
    w3e[:, :H3] = np.asarray(W3, np.float32).T
    b2hi, b2lo = _split10(np.asarray(b2, np.float32).reshape(1, H2))
    b3v = np.asarray(b3, np.float32).reshape(1, H3)
    b3hi, b3lo = _split10(b3v)
    # h3' cols 200,201 are the ones rows: psum = 1.0 there (hi row only)
    b3se = np.zeros((2, H3E), np.float32)
    b3se[0, :H3] = b3hi
    b3se[1, :H3] = b3lo
    b3se[0, H3:] = 1.0

    wmats = {
        "w1t": np.ascontiguousarray(np.asarray(W1, np.float32).T),
        "b1": np.asarray(b1, np.float32).reshape(H1, 1),
        "w2t": np.ascontiguousarray(np.asarray(W2, np.float32).T),
        "b2s": np.ascontiguousarray(np.vstack([b2hi, b2lo])),
        "w3t": np.ascontiguousarray(w3e),
        "b3s": np.ascontiguousarray(b3se),
        "wh": np.ascontiguousarray(WH),
        "onesr": np.ones((2, NB), np.float32),
    }

    in_maps = []
    for c in range(NCORES):
        sl = slice(c * BS, (c + 1) * BS)
        # batch row (tile t, col n=k*128+p) <- global row t*512 + p*4 + k
        xc = x0[sl].reshape(NT, 128, 4, 3).transpose(3, 0, 2, 1)
        rc = rand[sl].reshape(NT, 128, 4, D).transpose(1, 0, 2, 3)
        m = {
            "x0t": np.ascontiguousarray(xc.reshape(3, BS)),
            "gum": gumbel[sl].reshape(BS, GD),
            "rnd": np.ascontiguousarray(rc.reshape(128, NT * 16)),
        }
        m.update(wmats)
        in_maps.append(m)

    res = run_bass_kernel_spmd(nc, in_maps, core_ids=list(range(NCORES)))
    LAST_RESULT = res
    outs = []
    for c in range(NCORES):
        oc = res.results[c]["out"].reshape(128, NT, 4, D)
        outs.append(oc.transpose(1, 0, 2, 3).reshape(BS, D))
    return np.concatenate(outs, axis=0).astype(np.float32)
